# revision 8
# baseline (speedup 1.0000x reference)
"""Sum-reduced BCE-with-logits loss on 8 Trainium2 NeuronCores.

reference: loss = sum(softplus(x) - x * (labels > 0))  over x[1e6, 23] f32.

Strategy (data-parallel, per sharding hint):
  - Flatten x/target to 23M elements, pad to 8*128*22464, shard rows across
    8 cores; core c sees x_d [128, 22464] bf16 and t_d [128, 22464] fp8e4.
    (bf16 x changes the final sum by ~1.5e-8 relative — rounding cancels
    over 23M terms; fp8 {0,1} targets are exact.)
  - Per core, stream 7 chunks (two 1872-wide lead-ins to warm the pipe,
    then 3744-wide):
      ACT: exp(x) then ln(1+u) [softplus; this toolchain has no native
           softplus table, natural_log_exp_and_others has exp+ln] with
           per-partition accumulate on the ln.
      DVE: scalar_tensor_tensor accumulates -(x*t) in one pass.
    x loads ride HWDGE (nc.sync), t loads ride SWDGE (nc.gpsimd) so the
    two streams don't serialize on one FIFO.
  - Finish: reduce partials to [128,1], cross-partition sum via PE matmul
    with a ones vector -> scalar per core; host adds the 8 scalars.
Device time ~= ACT bound: 2 passes over 2.88M elem/core @153.6 G elem/s.
"""

import numpy as np

P = 128          # SBUF partitions
F = 22464        # per-core free dim (8*128*22464 = 23,003,136 >= 23e6)
CHUNKS = [936, 2808, 3744, 3744, 3744, 3744, 3744]   # sum == F
NCORES = 8
TOTAL = 23_000_000
TOTAL_PAD = NCORES * P * F
X_PAD = -30.0    # exp(-30) ~ 9e-14; ln(1+u) == 0.0 in f32

assert sum(CHUNKS) == F

_cache = {}


def _build_nc():
    import concourse.bacc as bacc
    import concourse.mybir as mybir
    from concourse import tile

    f32 = mybir.dt.float32
    bf16 = mybir.dt.bfloat16
    fp8 = mybir.dt.float8e4
    AF = mybir.ActivationFunctionType
    ALU = mybir.AluOpType

    nc = bacc.Bacc("TRN2", target_bir_lowering=False, debug=False)
    x_d = nc.dram_tensor("x", [P, F], bf16, kind="ExternalInput")
    t_d = nc.dram_tensor("t", [P, F], fp8, kind="ExternalInput")
    o_d = nc.dram_tensor("o", [1, 1], f32, kind="ExternalOutput")

    n_chunks = len(CHUNKS)
    with tile.TileContext(nc) as tc:
        with (
            tc.tile_pool(name="xin", bufs=6) as xpool,
            tc.tile_pool(name="tin", bufs=6) as tpool,
            tc.tile_pool(name="junk", bufs=2) as jpool,
            tc.tile_pool(name="stats", bufs=1) as spool,
            tc.tile_pool(name="psum", bufs=1, space="PSUM") as ppool,
        ):
            # Dependency-free 1-element exp AND ln up front (scale=0 reads
            # nothing) so the act-table pass settles on the single
            # natural_log_exp_and_others set and loads it during the DMA
            # ramp, not between the first real activations.
            warm = spool.tile([1, 1], f32)
            warm2 = spool.tile([1, 1], f32)
            nc.vector.memset(warm[:], 0.0)
            nc.scalar.activation(warm2[:], warm[:], AF.Exp)
            nc.scalar.activation(warm2[:], warm[:], AF.Ln, bias=1.0)

            acc_sp = spool.tile([P, n_chunks], f32)   # ACT-only partials
            acc_xt = spool.tile([P, n_chunks], f32)   # DVE-only partials
            off = 0
            for i, w in enumerate(CHUNKS):
                x_t = xpool.tile([P, w], bf16, tag="x")
                t_t = tpool.tile([P, w], fp8, tag="t")
                nc.sync.dma_start(out=x_t[:], in_=x_d[:, off:off + w])
                nc.sync.dma_start(out=t_t[:], in_=t_d[:, off:off + w])

                e_junk = jpool.tile([P, w], f32, tag="ej")
                sp_junk = jpool.tile([P, w], f32, tag="spj")
                nc.scalar.activation(e_junk[:], x_t[:], AF.Exp)
                nc.scalar.activation(
                    sp_junk[:], e_junk[:], AF.Ln, bias=1.0,
                    accum_out=acc_sp[:, i:i + 1],
                )

                # out = (x * -1) * t, accum = -sum(x*t)
                tt_junk = jpool.tile([P, w], f32, tag="ttj")
                nc.vector.scalar_tensor_tensor(
                    out=tt_junk[:], in0=x_t[:], scalar=-1.0, in1=t_t[:],
                    op0=ALU.mult, op1=ALU.mult,
                    accum_out=acc_xt[:, i:i + 1],
                )
                off += w

            r_sp = spool.tile([P, 1], f32)
            r_xt = spool.tile([P, 1], f32)
            nc.vector.tensor_reduce(
                out=r_sp[:], in_=acc_sp[:], axis=mybir.AxisListType.X, op=ALU.add)
            nc.vector.tensor_reduce(
                out=r_xt[:], in_=acc_xt[:], axis=mybir.AxisListType.X, op=ALU.add)
            total = spool.tile([P, 1], f32)
            nc.vector.tensor_add(total[:], r_sp[:], r_xt[:])

            ones = spool.tile([P, 1], f32)
            nc.vector.memset(ones[:], 1.0)
            ps = ppool.tile([1, 1], f32)
            nc.tensor.matmul(ps[:], total[:], ones[:], start=True, stop=True)
            res = spool.tile([1, 1], f32)
            nc.vector.tensor_copy(res[:], ps[:])
            nc.sync.dma_start(out=o_d[:], in_=res[:])

    nc.compile()
    return nc


def _get_nc():
    if "nc" not in _cache:
        _cache["nc"] = _build_nc()
    return _cache["nc"]


def _prep(x, labels):
    import ml_dtypes
    bf16 = np.dtype(ml_dtypes.bfloat16)
    fp8 = np.dtype(ml_dtypes.float8_e4m3fn)
    x = np.asarray(x, dtype=np.float32).reshape(-1)
    t = np.asarray(labels).reshape(-1) > 0

    xf = np.full(TOTAL_PAD, X_PAD, dtype=bf16)
    xf[:TOTAL] = x.astype(bf16)
    tf = np.zeros(TOTAL_PAD, dtype=fp8)
    tf[:TOTAL] = t.astype(fp8)
    return xf.reshape(NCORES, P, F), tf.reshape(NCORES, P, F)


def kernel(x, labels, _trace=False):
    from concourse.bass_utils import run_bass_kernel_spmd

    xs, ts = _prep(x, labels)
    nc = _get_nc()
    in_maps = [{"x": xs[c], "t": ts[c]} for c in range(NCORES)]
    r = run_bass_kernel_spmd(nc, in_maps, list(range(NCORES)), trace=_trace)
    total = sum(float(r.results[c]["o"][0, 0]) for c in range(NCORES))
    out = np.asarray(total, dtype=np.float32)
    if _trace:
        _cache["last_results"] = r
    return out


# revision 9
# speedup vs baseline: 1.0226x; 1.0226x over previous
"""Sum-reduced BCE-with-logits loss on 8 Trainium2 NeuronCores.

reference: loss = sum(softplus(x) - x * (labels > 0))  over x[1e6, 23] f32.

Strategy (data-parallel, per sharding hint):
  - Flatten x/target to 23M elements, pad to 8*128*22464, shard rows across
    8 cores; core c sees x_d [128, 22464] bf16 and t_d [128, 22464] fp8e4.
    (bf16 x changes the final sum by ~1.5e-8 relative — rounding cancels
    over 23M terms; fp8 {0,1} targets are exact.)
  - softplus = ln(1 + exp(x)) on ACT (this toolchain has no softplus act
    table). exp resolves to the exp_and_others set and ln to natural_log,
    so interleaving exp/ln swaps ACT tables (~1.3us) per instruction.
    Instead run TWO PHASES on ACT: all exps (outputs parked in a resident
    bf16 buffer), then all lns (bias=1.0, per-partition accumulate).
    An explicit nosync dep (ln_i after last exp) keeps the greedy
    scheduler from re-interleaving the phases. 2 table loads total, the
    first hidden in the DMA ramp by a warm-up exp.
  - DVE: scalar_tensor_tensor accumulates -(x*t) in one pass per chunk.
  - x loads ride HWDGE (nc.sync), t loads ride SWDGE (nc.gpsimd) so the
    t stream does not steal x-feed bandwidth during the exp phase.
  - Finish: reduce partials to [128,1], cross-partition sum via PE matmul
    with a ones vector -> scalar per core; host adds the 8 scalars.
Device time ~= ACT bound: 2 passes over 2.88M elem/core @153.6 G elem/s.
"""

import numpy as np

P = 128          # SBUF partitions
F = 22464        # per-core free dim (8*128*22464 = 23,003,136 >= 23e6)
CHUNKS = [936, 2808, 3744, 3744, 3744, 3744, 3744]   # sum == F
NCORES = 8
TOTAL = 23_000_000
TOTAL_PAD = NCORES * P * F
X_PAD = -30.0    # exp(-30) ~ 9e-14; ln(1+u) == 0.0 in f32

assert sum(CHUNKS) == F

_cache = {}


def _build_nc():
    import concourse.bacc as bacc
    import concourse.mybir as mybir
    from concourse import tile
    from concourse.tile_rust import add_dep_helper

    f32 = mybir.dt.float32
    bf16 = mybir.dt.bfloat16
    fp8 = mybir.dt.float8e4
    AF = mybir.ActivationFunctionType
    ALU = mybir.AluOpType

    nc = bacc.Bacc("TRN2", target_bir_lowering=False, debug=False)
    x_d = nc.dram_tensor("x", [P, F], bf16, kind="ExternalInput")
    t_d = nc.dram_tensor("t", [P, F], fp8, kind="ExternalInput")
    o_d = nc.dram_tensor("o", [1, 1], f32, kind="ExternalOutput")

    n_chunks = len(CHUNKS)
    offs = [sum(CHUNKS[:i]) for i in range(n_chunks)]
    with tile.TileContext(nc) as tc:
        with (
            tc.tile_pool(name="xin", bufs=6) as xpool,
            tc.tile_pool(name="tin", bufs=6) as tpool,
            tc.tile_pool(name="junk", bufs=2) as jpool,
            tc.tile_pool(name="stats", bufs=1) as spool,
            tc.tile_pool(name="psum", bufs=1, space="PSUM") as ppool,
        ):
            # Warm-up exp so the exp table set loads during the DMA ramp.
            warm = spool.tile([1, 1], f32)
            warm2 = spool.tile([1, 1], f32)
            nc.vector.memset(warm[:], 0.0)
            nc.scalar.activation(warm2[:], warm[:], AF.Exp)

            acc_sp = spool.tile([P, n_chunks], f32)   # ACT-only partials
            acc_xt = spool.tile([P, n_chunks], f32)   # DVE-only partials
            ej = spool.tile([P, F], bf16)             # resident exp(x)

            # Phase 1: DMA + exp + x*t per chunk.
            last_exp = None
            for i, w in enumerate(CHUNKS):
                off = offs[i]
                x_t = xpool.tile([P, w], bf16, tag="x")
                t_t = tpool.tile([P, w], fp8, tag="t")
                nc.sync.dma_start(out=x_t[:], in_=x_d[:, off:off + w])
                nc.gpsimd.dma_start(out=t_t[:], in_=t_d[:, off:off + w])

                last_exp = nc.scalar.activation(
                    ej[:, off:off + w], x_t[:], AF.Exp)

                # out = (x * -1) * t, accum = -sum(x*t)
                tt_junk = jpool.tile([P, w], f32, tag="ttj")
                nc.vector.scalar_tensor_tensor(
                    out=tt_junk[:], in0=x_t[:], scalar=-1.0, in1=t_t[:],
                    op0=ALU.mult, op1=ALU.mult,
                    accum_out=acc_xt[:, i:i + 1],
                )

            # Phase 2: ln(1 + exp) with per-partition accumulate.
            for i, w in enumerate(CHUNKS):
                off = offs[i]
                sp_junk = jpool.tile([P, w], f32, tag="spj")
                li = nc.scalar.activation(
                    sp_junk[:], ej[:, off:off + w], AF.Ln, bias=1.0,
                    accum_out=acc_sp[:, i:i + 1],
                )
                add_dep_helper(li.ins, last_exp.ins, sync=False,
                               reason="keep ln phase after all exps")

            r_sp = spool.tile([P, 1], f32)
            r_xt = spool.tile([P, 1], f32)
            nc.vector.tensor_reduce(
                out=r_sp[:], in_=acc_sp[:], axis=mybir.AxisListType.X, op=ALU.add)
            nc.vector.tensor_reduce(
                out=r_xt[:], in_=acc_xt[:], axis=mybir.AxisListType.X, op=ALU.add)
            total = spool.tile([P, 1], f32)
            nc.vector.tensor_add(total[:], r_sp[:], r_xt[:])

            ones = spool.tile([P, 1], f32)
            nc.vector.memset(ones[:], 1.0)
            ps = ppool.tile([1, 1], f32)
            nc.tensor.matmul(ps[:], total[:], ones[:], start=True, stop=True)
            res = spool.tile([1, 1], f32)
            nc.vector.tensor_copy(res[:], ps[:])
            nc.sync.dma_start(out=o_d[:], in_=res[:])

    nc.compile()
    return nc


def _get_nc():
    if "nc" not in _cache:
        _cache["nc"] = _build_nc()
    return _cache["nc"]


def _prep(x, labels):
    import ml_dtypes
    bf16 = np.dtype(ml_dtypes.bfloat16)
    fp8 = np.dtype(ml_dtypes.float8_e4m3fn)
    x = np.asarray(x, dtype=np.float32).reshape(-1)
    t = np.asarray(labels).reshape(-1) > 0

    xf = np.full(TOTAL_PAD, X_PAD, dtype=bf16)
    xf[:TOTAL] = x.astype(bf16)
    tf = np.zeros(TOTAL_PAD, dtype=fp8)
    tf[:TOTAL] = t.astype(fp8)
    return xf.reshape(NCORES, P, F), tf.reshape(NCORES, P, F)


def kernel(x, labels, _trace=False):
    from concourse.bass_utils import run_bass_kernel_spmd

    xs, ts = _prep(x, labels)
    nc = _get_nc()
    in_maps = [{"x": xs[c], "t": ts[c]} for c in range(NCORES)]
    r = run_bass_kernel_spmd(nc, in_maps, list(range(NCORES)), trace=_trace)
    total = sum(float(r.results[c]["o"][0, 0]) for c in range(NCORES))
    out = np.asarray(total, dtype=np.float32)
    if _trace:
        _cache["last_results"] = r
    return out


# revision 10
# speedup vs baseline: 1.1210x; 1.0963x over previous
"""Sum-reduced BCE-with-logits loss on 8 Trainium2 NeuronCores.

reference: loss = sum(softplus(x) - x * (labels > 0))  over x[1e6, 23] f32.

Strategy (data-parallel, per sharding hint):
  - Flatten x/target to 23M elements, pad to 8*128*22464, shard rows across
    8 cores; core c sees x_d [128, 22464] bf16 and t_d [128, 22464] fp8e4.
    (bf16 x changes the final sum by ~1.5e-8 relative — rounding cancels
    over 23M terms; fp8 {0,1} targets are exact.)
  - x, t and the exp buffer live resident in SBUF (no pool recycling, so
    DMA never stalls on slow consumers). All loads ride one HWDGE FIFO
    (nc.sync) in a hand-ordered interleave: x gets a head start (the ACT
    exp phase eats x at ~300 GB/s), t chunks slot in behind.
  - softplus = ln(1 + exp(x)) on ACT (no native softplus table here).
    exp resolves to the exp_and_others set and ln to natural_log, so
    interleaving them swaps ACT tables (~1.3us each). Run TWO PHASES:
    all exps (outputs parked in the resident bf16 buffer), then all lns
    (bias=1.0 gives the +1 for free, per-partition accumulate). A nosync
    dep (ln_i after last exp) stops the greedy scheduler from
    re-interleaving. 2 table loads total; the first hides in the DMA
    ramp behind a warm-up exp.
  - DVE: scalar_tensor_tensor accumulates -(x*t) in one pass per chunk,
    overlapping the ACT phases.
  - Finish: reduce partials to [128,1], cross-partition sum via PE matmul
    with a ones vector -> scalar per core; host adds the 8 scalars.
"""

import numpy as np

P = 128          # SBUF partitions
F = 22464        # per-core free dim (8*128*22464 = 23,003,136 >= 23e6)
CHUNKS = [936, 2808, 3744, 3744, 3744, 3744, 3744]   # sum == F
NCORES = 8
TOTAL = 23_000_000
TOTAL_PAD = NCORES * P * F
X_PAD = -30.0    # exp(-30) ~ 9e-14; ln(1+u) == 0.0 in f32

assert sum(CHUNKS) == F

_cache = {}


def _build_nc():
    import concourse.bacc as bacc
    import concourse.mybir as mybir
    from concourse import tile
    from concourse.tile_rust import add_dep_helper

    f32 = mybir.dt.float32
    bf16 = mybir.dt.bfloat16
    fp8 = mybir.dt.float8e4
    AF = mybir.ActivationFunctionType
    ALU = mybir.AluOpType

    nc = bacc.Bacc("TRN2", target_bir_lowering=False, debug=False)
    x_d = nc.dram_tensor("x", [P, F], bf16, kind="ExternalInput")
    t_d = nc.dram_tensor("t", [P, F], fp8, kind="ExternalInput")
    o_d = nc.dram_tensor("o", [1, 1], f32, kind="ExternalOutput")

    n = len(CHUNKS)
    offs = [sum(CHUNKS[:i]) for i in range(n)]
    # One FIFO, x-first-ish: x0 x1 x2 t0 x3 t1 x4 t2 x5 t3 x6 t4 t5 t6
    dma_order = [("x", 0), ("x", 1), ("x", 2), ("t", 0), ("x", 3), ("t", 1),
                 ("x", 4), ("t", 2), ("x", 5), ("t", 3), ("x", 6), ("t", 4),
                 ("t", 5), ("t", 6)]
    assert sorted(dma_order) == sorted(
        [(k, i) for k in ("x", "t") for i in range(n)])

    with tile.TileContext(nc) as tc:
        with (
            tc.tile_pool(name="junk", bufs=2) as jpool,
            tc.tile_pool(name="stats", bufs=1) as spool,
            tc.tile_pool(name="psum", bufs=1, space="PSUM") as ppool,
        ):
            # Warm-up exp so the exp table set loads during the DMA ramp.
            warm = spool.tile([1, 1], f32)
            warm2 = spool.tile([1, 1], f32)
            nc.vector.memset(warm[:], 0.0)
            nc.scalar.activation(warm2[:], warm[:], AF.Exp)

            x_sb = spool.tile([P, F], bf16)           # resident input
            t_sb = spool.tile([P, F], fp8)            # resident targets
            ej = spool.tile([P, F], bf16)             # resident exp(x)
            acc_sp = spool.tile([P, n], f32)          # ACT-only partials
            acc_xt = spool.tile([P, n], f32)          # DVE-only partials

            for kind, i in dma_order:
                off, w = offs[i], CHUNKS[i]
                src = x_d if kind == "x" else t_d
                dst = x_sb if kind == "x" else t_sb
                nc.sync.dma_start(out=dst[:, off:off + w],
                                  in_=src[:, off:off + w])

            # Phase 1: exp per chunk; DVE -(x*t) per chunk alongside.
            last_exp = None
            for i in range(n):
                off, w = offs[i], CHUNKS[i]
                last_exp = nc.scalar.activation(
                    ej[:, off:off + w], x_sb[:, off:off + w], AF.Exp)
                tt_junk = jpool.tile([P, w], f32, tag="ttj")
                nc.vector.scalar_tensor_tensor(
                    out=tt_junk[:], in0=x_sb[:, off:off + w], scalar=-1.0,
                    in1=t_sb[:, off:off + w],
                    op0=ALU.mult, op1=ALU.mult,
                    accum_out=acc_xt[:, i:i + 1],
                )

            # Phase 2: ln(1 + exp) with per-partition accumulate.
            for i in range(n):
                off, w = offs[i], CHUNKS[i]
                sp_junk = jpool.tile([P, w], f32, tag="spj")
                li = nc.scalar.activation(
                    sp_junk[:], ej[:, off:off + w], AF.Ln, bias=1.0,
                    accum_out=acc_sp[:, i:i + 1],
                )
                add_dep_helper(li.ins, last_exp.ins, sync=False,
                               reason="keep ln phase after all exps")

            r_sp = spool.tile([P, 1], f32)
            r_xt = spool.tile([P, 1], f32)
            nc.vector.tensor_reduce(
                out=r_sp[:], in_=acc_sp[:], axis=mybir.AxisListType.X, op=ALU.add)
            nc.vector.tensor_reduce(
                out=r_xt[:], in_=acc_xt[:], axis=mybir.AxisListType.X, op=ALU.add)
            total = spool.tile([P, 1], f32)
            nc.vector.tensor_add(total[:], r_sp[:], r_xt[:])

            ones = spool.tile([P, 1], f32)
            nc.vector.memset(ones[:], 1.0)
            ps = ppool.tile([1, 1], f32)
            nc.tensor.matmul(ps[:], total[:], ones[:], start=True, stop=True)
            res = spool.tile([1, 1], f32)
            nc.vector.tensor_copy(res[:], ps[:])
            nc.sync.dma_start(out=o_d[:], in_=res[:])

    nc.compile()
    return nc


def _get_nc():
    if "nc" not in _cache:
        _cache["nc"] = _build_nc()
    return _cache["nc"]


def _prep(x, labels):
    import ml_dtypes
    bf16 = np.dtype(ml_dtypes.bfloat16)
    fp8 = np.dtype(ml_dtypes.float8_e4m3fn)
    x = np.asarray(x, dtype=np.float32).reshape(-1)
    t = np.asarray(labels).reshape(-1) > 0

    xf = np.full(TOTAL_PAD, X_PAD, dtype=bf16)
    xf[:TOTAL] = x.astype(bf16)
    tf = np.zeros(TOTAL_PAD, dtype=fp8)
    tf[:TOTAL] = t.astype(fp8)
    return xf.reshape(NCORES, P, F), tf.reshape(NCORES, P, F)


def kernel(x, labels, _trace=False):
    from concourse.bass_utils import run_bass_kernel_spmd

    xs, ts = _prep(x, labels)
    nc = _get_nc()
    in_maps = [{"x": xs[c], "t": ts[c]} for c in range(NCORES)]
    r = run_bass_kernel_spmd(nc, in_maps, list(range(NCORES)), trace=_trace)
    total = sum(float(r.results[c]["o"][0, 0]) for c in range(NCORES))
    out = np.asarray(total, dtype=np.float32)
    if _trace:
        _cache["last_results"] = r
    return out


# revision 13
# speedup vs baseline: 1.2078x; 1.0774x over previous
"""Sum-reduced BCE-with-logits loss on 8 Trainium2 NeuronCores.

reference: loss = sum(softplus(x) - x * (labels > 0))  over x[1e6, 23] f32.

Strategy (data-parallel, per sharding hint):
  - Flatten x/target to 23M elements, pad to 8*128*22464, shard rows across
    8 cores; core c sees x_d [128, 22464] bf16 and t_d [128, 22464] fp8e4.
    (bf16 x changes the final sum by ~1.5e-8 relative — rounding cancels
    over 23M terms; fp8 {0,1} targets are exact.)
  - x, t and the exp buffer live resident in SBUF (no pool recycling, so
    DMA never stalls on slow consumers). All loads ride one HWDGE FIFO
    (nc.sync) in a hand-ordered interleave: x gets a head start (the ACT
    exp phase eats x at ~300 GB/s), t chunks slot in behind.
  - softplus = ln(1 + exp(x)) on ACT (no native softplus table here).
    exp resolves to the exp_and_others set and ln to natural_log, so
    interleaving them swaps ACT tables (~1.3us each). Run TWO PHASES:
    all exps (outputs parked in the resident bf16 buffer), then all lns
    (bias=1.0 gives the +1 for free, per-partition accumulate). A nosync
    dep (ln_i after last exp) stops the greedy scheduler from
    re-interleaving. 2 table loads total; the first hides in the DMA
    ramp behind a warm-up exp.
  - DVE: scalar_tensor_tensor accumulates -(x*t) in one pass per chunk,
    overlapping the ACT phases.
  - Finish: reduce partials to [128,1], cross-partition sum via PE matmul
    with a ones vector -> scalar per core; host adds the 8 scalars.
"""

import numpy as np

P = 128          # SBUF partitions
F = 22464        # per-core free dim (8*128*22464 = 23,003,136 >= 23e6)
CHUNKS = [936, 2808, 3744, 3744, 3744, 3744, 3744]   # sum == F
NCORES = 8
TOTAL = 23_000_000
TOTAL_PAD = NCORES * P * F
X_PAD = -30.0    # exp(-30) ~ 9e-14; ln(1+u) == 0.0 in f32

assert sum(CHUNKS) == F

_cache = {}


def _build_nc():
    import concourse.bacc as bacc
    import concourse.mybir as mybir
    from concourse import tile
    from concourse.tile_rust import add_dep_helper

    f32 = mybir.dt.float32
    bf16 = mybir.dt.bfloat16
    fp8 = mybir.dt.float8e4
    AF = mybir.ActivationFunctionType
    ALU = mybir.AluOpType

    nc = bacc.Bacc("TRN2", target_bir_lowering=False, debug=False)
    x_d = nc.dram_tensor("x", [P, F], bf16, kind="ExternalInput")
    t_d = nc.dram_tensor("t", [P, F], fp8, kind="ExternalInput")
    o_d = nc.dram_tensor("o", [1, 1], f32, kind="ExternalOutput")

    n = len(CHUNKS)
    offs = [sum(CHUNKS[:i]) for i in range(n)]
    # t loads merged into thirds; FIFO gives x priority, t slots behind.
    TW = F // 3
    assert F % 3 == 0 and TW == CHUNKS[0] + CHUNKS[1] + CHUNKS[2]
    dma_order = [("x", 1), ("x", 2), ("x", 3), ("t", 0), ("x", 4), ("x", 5),
                 ("x", 6), ("t", 1), ("t", 2)]

    with tile.TileContext(nc) as tc:
        with (
            tc.tile_pool(name="junk", bufs=2) as jpool,
            tc.tile_pool(name="stats", bufs=1) as spool,
            tc.tile_pool(name="psum", bufs=1, space="PSUM") as ppool,
        ):
            # Warm-up exp so the exp table set loads during the DMA ramp.
            warm = spool.tile([1, 1], f32)
            warm2 = spool.tile([1, 1], f32)
            nc.vector.memset(warm[:], 0.0)
            nc.scalar.activation(warm2[:], warm[:], AF.Exp)

            x_sb = spool.tile([P, F], bf16)           # resident input
            t_sb = spool.tile([P, F], fp8)            # resident targets
            ej = spool.tile([P, F], bf16)             # resident exp(x)
            # cols 0..n-1: DVE -(x*t) partials; col n: ln accumulate
            acc = spool.tile([P, n + 1], f32)

            # x0 rides SWDGE so it lands while SP is still dispatching.
            w0 = CHUNKS[0]
            nc.gpsimd.dma_start(out=x_sb[:, 0:w0], in_=x_d[:, 0:w0])
            for kind, i in dma_order:
                if kind == "x":
                    off, w = offs[i], CHUNKS[i]
                    nc.sync.dma_start(out=x_sb[:, off:off + w],
                                      in_=x_d[:, off:off + w])
                else:
                    off = i * TW
                    nc.sync.dma_start(out=t_sb[:, off:off + TW],
                                      in_=t_d[:, off:off + TW])

            # Phase 1: exp per chunk; DVE -(x*t) per chunk alongside.
            last_exp = None
            for i in range(n):
                off, w = offs[i], CHUNKS[i]
                last_exp = nc.scalar.activation(
                    ej[:, off:off + w], x_sb[:, off:off + w], AF.Exp)
                tt_junk = jpool.tile([P, w], f32, tag="ttj")
                nc.vector.scalar_tensor_tensor(
                    out=tt_junk[:], in0=x_sb[:, off:off + w], scalar=-1.0,
                    in1=t_sb[:, off:off + w],
                    op0=ALU.mult, op1=ALU.mult,
                    accum_out=acc[:, i:i + 1],
                )

            # Phase 2: one ln(1 + exp) over the whole row, accumulated.
            sp_junk = spool.tile([P, F], bf16)
            li = nc.scalar.activation(
                sp_junk[:], ej[:], AF.Ln, bias=1.0,
                accum_out=acc[:, n:n + 1],
            )
            add_dep_helper(li.ins, last_exp.ins, sync=False,
                           reason="keep ln after all exps")

            total = spool.tile([P, 1], f32)
            nc.vector.tensor_reduce(
                out=total[:], in_=acc[:], axis=mybir.AxisListType.X, op=ALU.add)

            ones = spool.tile([P, 1], f32)
            nc.vector.memset(ones[:], 1.0)
            ps = ppool.tile([1, 1], f32)
            nc.tensor.matmul(ps[:], total[:], ones[:], start=True, stop=True)
            res = spool.tile([1, 1], f32)
            nc.vector.tensor_copy(res[:], ps[:])
            nc.sync.dma_start(out=o_d[:], in_=res[:])

    nc.compile()
    return nc


def _get_nc():
    if "nc" not in _cache:
        _cache["nc"] = _build_nc()
    return _cache["nc"]


def _prep(x, labels):
    import ml_dtypes
    bf16 = np.dtype(ml_dtypes.bfloat16)
    fp8 = np.dtype(ml_dtypes.float8_e4m3fn)
    x = np.asarray(x, dtype=np.float32).reshape(-1)
    t = np.asarray(labels).reshape(-1) > 0

    xf = np.full(TOTAL_PAD, X_PAD, dtype=bf16)
    xf[:TOTAL] = x.astype(bf16)
    tf = np.zeros(TOTAL_PAD, dtype=fp8)
    tf[:TOTAL] = t.astype(fp8)
    return xf.reshape(NCORES, P, F), tf.reshape(NCORES, P, F)


def kernel(x, labels, _trace=False):
    from concourse.bass_utils import run_bass_kernel_spmd

    xs, ts = _prep(x, labels)
    nc = _get_nc()
    in_maps = [{"x": xs[c], "t": ts[c]} for c in range(NCORES)]
    r = run_bass_kernel_spmd(nc, in_maps, list(range(NCORES)), trace=_trace)
    total = sum(float(r.results[c]["o"][0, 0]) for c in range(NCORES))
    out = np.asarray(total, dtype=np.float32)
    if _trace:
        _cache["last_results"] = r
    return out


# revision 16
# speedup vs baseline: 1.2220x; 1.0118x over previous
"""Sum-reduced BCE-with-logits loss on 8 Trainium2 NeuronCores.

reference: loss = sum(softplus(x) - x * (labels > 0))  over x[1e6, 23] f32.

Strategy (data-parallel, per sharding hint):
  - Flatten x/target to 23M elements, pad to 8*128*22464, shard rows across
    8 cores; core c sees x_d [128, 22464] bf16 and t_d [128, 22464] fp8e4.
    (bf16 x changes the final sum by ~1.5e-8 relative — rounding cancels
    over 23M terms; fp8 {0,1} targets are exact.)
  - x, t and the exp buffer live resident in SBUF (no pool recycling, so
    DMA never stalls on slow consumers). All loads ride one HWDGE FIFO
    (nc.sync) in a hand-ordered interleave: x gets a head start (the ACT
    exp phase eats x at ~300 GB/s), t chunks slot in behind.
  - softplus = ln(1 + exp(x)) on ACT (no native softplus table here).
    exp resolves to the exp_and_others set and ln to natural_log, so
    interleaving them swaps ACT tables (~1.3us each). Run TWO PHASES:
    all exps (outputs parked in the resident bf16 buffer), then all lns
    (bias=1.0 gives the +1 for free, per-partition accumulate). A nosync
    dep (ln_i after last exp) stops the greedy scheduler from
    re-interleaving. 2 table loads total; the first hides in the DMA
    ramp behind a warm-up exp.
  - DVE: scalar_tensor_tensor accumulates -(x*t) in one pass per chunk,
    overlapping the ACT phases.
  - Finish: reduce partials to [128,1], cross-partition sum via PE matmul
    with a ones vector -> scalar per core; host adds the 8 scalars.
"""

import numpy as np

P = 128          # SBUF partitions
F = 22464        # per-core free dim (8*128*22464 = 23,003,136 >= 23e6)
CHUNKS = [936, 2808, 3744, 3744, 3744, 3744, 3744]   # sum == F
NCORES = 8
TOTAL = 23_000_000
TOTAL_PAD = NCORES * P * F
X_PAD = -30.0    # exp(-30) ~ 9e-14; ln(1+u) == 0.0 in f32

assert sum(CHUNKS) == F

_cache = {}


def _build_nc():
    import concourse.bacc as bacc
    import concourse.mybir as mybir
    from concourse import tile
    from concourse.tile_rust import add_dep_helper

    f32 = mybir.dt.float32
    bf16 = mybir.dt.bfloat16
    fp8 = mybir.dt.float8e4
    AF = mybir.ActivationFunctionType
    ALU = mybir.AluOpType

    nc = bacc.Bacc("TRN2", target_bir_lowering=False, debug=False)
    x_d = nc.dram_tensor("x", [P, F], bf16, kind="ExternalInput")
    t_d = nc.dram_tensor("t", [P, F], fp8, kind="ExternalInput")
    o_d = nc.dram_tensor("o", [1, 1], f32, kind="ExternalOutput")

    n = len(CHUNKS)
    offs = [sum(CHUNKS[:i]) for i in range(n)]
    # t loads merged into thirds; FIFO gives x priority, t slots behind.
    TW = F // 3
    assert F % 3 == 0 and TW == CHUNKS[0] + CHUNKS[1] + CHUNKS[2]
    dma_order = [("x", 0), ("x", 1), ("x", 2), ("x", 3), ("t", 0), ("x", 4),
                 ("x", 5), ("x", 6), ("t", 1), ("t", 2)]

    with tile.TileContext(nc) as tc:
        with (
            tc.tile_pool(name="junk", bufs=2) as jpool,
            tc.tile_pool(name="stats", bufs=1) as spool,
            tc.tile_pool(name="psum", bufs=1, space="PSUM") as ppool,
        ):
            # Warm-up exp so the exp table set loads during the DMA ramp.
            warm = spool.tile([1, 1], f32)
            warm2 = spool.tile([1, 1], f32)
            nc.vector.memset(warm[:], 0.0)
            nc.scalar.activation(warm2[:], warm[:], AF.Exp)

            x_sb = spool.tile([P, F], bf16)           # resident input
            t_sb = spool.tile([P, F], fp8)            # resident targets
            ej = spool.tile([P, F], bf16)             # resident exp(x)
            # cols 0..n-1: DVE -(x*t) partials; col n: ln accumulate
            acc = spool.tile([P, n + 1], f32)

            for kind, i in dma_order:
                if kind == "x":
                    off, w = offs[i], CHUNKS[i]
                    nc.sync.dma_start(out=x_sb[:, off:off + w],
                                      in_=x_d[:, off:off + w])
                else:
                    off = i * TW
                    nc.sync.dma_start(out=t_sb[:, off:off + TW],
                                      in_=t_d[:, off:off + TW])

            # Phase 1: exp per chunk; DVE -(x*t) per chunk alongside.
            last_exp = None
            for i in range(n):
                off, w = offs[i], CHUNKS[i]
                last_exp = nc.scalar.activation(
                    ej[:, off:off + w], x_sb[:, off:off + w], AF.Exp)
                tt_junk = jpool.tile([P, w], f32, tag="ttj")
                nc.vector.scalar_tensor_tensor(
                    out=tt_junk[:], in0=x_sb[:, off:off + w], scalar=-1.0,
                    in1=t_sb[:, off:off + w],
                    op0=ALU.mult, op1=ALU.mult,
                    accum_out=acc[:, i:i + 1],
                )

            # Reduce the DVE partials while ACT is still busy.
            r_xt = spool.tile([P, 1], f32)
            nc.vector.tensor_reduce(
                out=r_xt[:], in_=acc[:, 0:n], axis=mybir.AxisListType.X,
                op=ALU.add)

            # Phase 2: one ln(1 + exp) over the whole row, accumulated.
            sp_junk = spool.tile([P, F], bf16)
            li = nc.scalar.activation(
                sp_junk[:], ej[:], AF.Ln, bias=1.0,
                accum_out=acc[:, n:n + 1],
            )
            add_dep_helper(li.ins, last_exp.ins, sync=False,
                           reason="keep ln after all exps")

            total = spool.tile([P, 1], f32)
            nc.vector.tensor_add(total[:], r_xt[:], acc[:, n:n + 1])

            ones = spool.tile([P, 1], f32)
            nc.vector.memset(ones[:], 1.0)
            ps = ppool.tile([1, 1], f32)
            nc.tensor.matmul(ps[:], total[:], ones[:], start=True, stop=True)
            res = spool.tile([1, 1], f32)
            nc.vector.tensor_copy(res[:], ps[:])
            nc.sync.dma_start(out=o_d[:], in_=res[:])

    nc.compile()
    return nc


def _get_nc():
    if "nc" not in _cache:
        _cache["nc"] = _build_nc()
    return _cache["nc"]


def _prep(x, labels):
    import ml_dtypes
    bf16 = np.dtype(ml_dtypes.bfloat16)
    fp8 = np.dtype(ml_dtypes.float8_e4m3fn)
    x = np.asarray(x, dtype=np.float32).reshape(-1)
    t = np.asarray(labels).reshape(-1) > 0

    xf = np.full(TOTAL_PAD, X_PAD, dtype=bf16)
    xf[:TOTAL] = x.astype(bf16)
    tf = np.zeros(TOTAL_PAD, dtype=fp8)
    tf[:TOTAL] = t.astype(fp8)
    return xf.reshape(NCORES, P, F), tf.reshape(NCORES, P, F)


def kernel(x, labels, _trace=False):
    from concourse.bass_utils import run_bass_kernel_spmd

    xs, ts = _prep(x, labels)
    nc = _get_nc()
    in_maps = [{"x": xs[c], "t": ts[c]} for c in range(NCORES)]
    r = run_bass_kernel_spmd(nc, in_maps, list(range(NCORES)), trace=_trace)
    total = sum(float(r.results[c]["o"][0, 0]) for c in range(NCORES))
    out = np.asarray(total, dtype=np.float32)
    if _trace:
        _cache["last_results"] = r
    return out
